# revision 5
# baseline (speedup 1.0000x reference)
"""Cross-attention layer on 8 TRN2 NeuronCores.

Sharding: core i -> (batch b = i//2, head-group g = i%2); each core computes
its head-group's contribution to out[b] through Wo; the host sums the two
partial products per batch (row-split of Wo => partial-sum reduction).

Device kernel works in transposed layout ([channels, tokens]) so the softmax
reduction is along the matmul free axis:
  Q^T = Wq_g^T x^T, K^T = Wk_g^T ctx^T, V = ctx Wv_g (+ ones column)
  scores^T_h = K_h Q_h^T  (contraction over head_dim=64)
  E = exp(scores^T/32) * mask^T      (no max subtraction; |scores/32| ~ 1.5)
  U = V'^T E  (per s-tile accumulation; row 64 = softmax denominator)
  O^T = U[0:64] * recip(U[64])       (DVE reciprocal + gpsimd broadcast)
  out_partial = O^T^T Wo_g           (host adds core pairs)

Issue order is software-pipelined: attention units start right after the
first K/Q column chunks are projected; remaining projections, the V
projection and the out-projection (first t-half) run on the PE underneath
the ACT exp chain, which is the stage-C pacer.  The reciprocal runs on DVE
(no ACT Exp<->Ln table swaps, which cost ~40us and re-throttled the PE).

Dtype split: x/ctx/Wq/Wk/Wv, Q^T/K^T/V', probs run in bf16 (pre-softmax
noise is negligible, probs/V noise ~0.5%); U accumulates in fp32 PSUM; the
normalized O^T and the Wo projection run in float32r (~1e-4).
"""

import os
import numpy as np
import ml_dtypes

import concourse.mybir as mybir
from concourse import bacc
import concourse.tile as tile
from concourse.bass_utils import run_bass_kernel_spmd

B, T, TC = 4, 1024, 1024
C, CTX_C, H = 1024, 1024, 16
HD = C // H            # 64
P = 128
NCORES = 8
HG = 2                 # head groups
HPG = H // HG          # 8 heads per core
CG = HPG * HD          # 512 channels per group
NT = 512               # matmul free-dim chunk
KO = C // P            # 8 contraction tiles for projections
MQ = CG // P           # 4 partition-tiles of Q^T/K^T
SO = TC // P           # 8 s-tiles
T2 = T // NT           # 2 t-chunks
KP = CG // P           # 4 contraction tiles for the out projection
F32 = mybir.dt.float32
F32R = mybir.dt.float32r
BF16 = mybir.dt.bfloat16
ALU = mybir.AluOpType
ACTF = mybir.ActivationFunctionType

_CACHED_NC = None


def _ensure_ntff_hook():
    """Register the axon NTFF profiling hook if the image's antenv lacks it."""
    try:
        from antenv.axon_hooks import get_axon_ntff_profile_hook  # noqa: F401
        return
    except ImportError:
        pass
    import sys
    import types
    try:
        from trn_agent_boot.trn_boot import _ntff_profile_via_ctypes
        hook = _ntff_profile_via_ctypes("/opt/axon/libaxon_pjrt.so")
    except Exception:
        hook = None
    mod = types.ModuleType("antenv.axon_hooks")
    mod.get_axon_ntff_profile_hook = lambda: hook
    mod.set_axon_ntff_profile_hook = lambda h: None
    sys.modules["antenv.axon_hooks"] = mod
    import antenv
    antenv.axon_hooks = mod


def _hp(h):
    """Partition slice of local head h inside a [128, MQ, ...] channel tile."""
    lo = (h % 2) * HD
    return slice(lo, lo + HD)


def _build_program():
    nc = bacc.Bacc("TRN2", target_bir_lowering=False, debug=False,
                   num_devices=NCORES)
    xT = nc.dram_tensor("xT", [C, T], BF16, kind="ExternalInput").ap()
    ctxT = nc.dram_tensor("ctxT", [CTX_C, TC], BF16, kind="ExternalInput").ap()
    maskT = nc.dram_tensor("maskT", [TC, T], BF16, kind="ExternalInput").ap()
    wq = nc.dram_tensor("wq", [C, CG], BF16, kind="ExternalInput").ap()
    wk = nc.dram_tensor("wk", [CTX_C, CG], BF16, kind="ExternalInput").ap()
    wv = nc.dram_tensor("wv", [CTX_C, CG], BF16, kind="ExternalInput").ap()
    wo = nc.dram_tensor("wo", [CG, C], F32, kind="ExternalInput").ap()
    out = nc.dram_tensor("out", [T, C], F32, kind="ExternalOutput").ap()

    with tile.TileContext(nc) as tc:
        with (
            tc.tile_pool(name="persist", bufs=1) as persist,
            tc.tile_pool(name="work", bufs=3) as work,
            tc.tile_pool(name="psmm", bufs=2, space="PSUM") as psmm,
            tc.tile_pool(name="pssc", bufs=2, space="PSUM") as pssc,
            tc.tile_pool(name="psu", bufs=2, space="PSUM") as psu_pool,
        ):
            qt_sb = persist.tile([P, MQ, T], BF16)            # Q^T [(h,d), t]
            kt_sb = persist.tile([P, MQ, TC], BF16)           # K^T [(h,d), s]
            vp_sb = persist.tile([P, SO, HPG, HD + 1], BF16)  # V' + ones col
            mask_sb = persist.tile([P, SO, T], BF16)          # mask^T
            ot_sb = persist.tile([P, KP, T], F32R)            # O^T normalized
            wo_sb = persist.tile([P, KP, C], F32R)
            xT_sb = persist.tile([P, KO, T], BF16)
            ctxT_sb = persist.tile([P, KO, TC], BF16)
            wq_sb = persist.tile([P, KO, CG], BF16)
            wk_sb = persist.tile([P, KO, CG], BF16)
            wv_sb = persist.tile([P, KO, CG], BF16)

            nc.gpsimd.memset(vp_sb[:, :, :, HD:HD + 1], 1.0)

            xT_r = xT.rearrange("(ko p) t -> p ko t", p=P)
            ctxT_r = ctxT.rearrange("(ko p) t -> p ko t", p=P)
            wq_r = wq.rearrange("(ko p) m -> p ko m", p=P)
            wk_r = wk.rearrange("(ko p) m -> p ko m", p=P)
            mask_r = maskT.rearrange("(so p) t -> p so t", p=P)

            def _mcols(m):
                return slice(m * P, (m + 1) * P)

            # DMAs in consumption order. B(0) needs wk[m0]+ctxT; V needs
            # wv+ctxT; A(0) needs wq[m0]+xT[t2=0]; first mask unit follows.
            nc.sync.dma_start(wk_sb[:, :, 0:P], wk_r[:, :, 0:P])
            for kc in range(KO):
                nc.sync.dma_start(ctxT_sb[:, kc], ctxT_r[:, kc])
            nc.sync.dma_start(wv_sb, wv.rearrange("(ko p) m -> p ko m", p=P))
            nc.sync.dma_start(wq_sb[:, :, 0:P], wq_r[:, :, 0:P])
            for kc in range(KO):
                nc.sync.dma_start(xT_sb[:, kc, 0:NT], xT_r[:, kc, 0:NT])
            nc.sync.dma_start(wk_sb[:, :, _mcols(1)], wk_r[:, :, _mcols(1)])
            nc.sync.dma_start(wq_sb[:, :, _mcols(1)], wq_r[:, :, _mcols(1)])
            nc.sync.dma_start(mask_sb[:, 0:2], mask_r[:, 0:2])
            for m in range(2, MQ):
                nc.sync.dma_start(wk_sb[:, :, _mcols(m)], wk_r[:, :, _mcols(m)])
                nc.sync.dma_start(wq_sb[:, :, _mcols(m)], wq_r[:, :, _mcols(m)])
            nc.sync.dma_start(mask_sb[:, 2:4], mask_r[:, 2:4])
            for kc in range(KO):
                nc.sync.dma_start(xT_sb[:, kc, NT:T], xT_r[:, kc, NT:T])
            for j in range(2, SO // 2):
                nc.sync.dma_start(mask_sb[:, 2 * j:2 * j + 2],
                                  mask_r[:, 2 * j:2 * j + 2])
            nc.sync.dma_start(
                wo_sb, wo.rearrange("(ko p) n -> p ko n", p=P).bitcast(F32R))

            # ---- projection building blocks ----
            def proj_B(m):           # K^T chunk m = Wk[:,mcols]^T ctx^T
                for s2 in range(T2):
                    ps = psmm.tile([P, NT], F32, tag="mm512")
                    for kc in range(KO):
                        nc.tensor.matmul(
                            ps, wk_sb[:, kc, _mcols(m)],
                            ctxT_sb[:, kc, s2 * NT:(s2 + 1) * NT],
                            start=(kc == 0), stop=(kc == KO - 1))
                    nc.vector.tensor_copy(
                        kt_sb[:, m, s2 * NT:(s2 + 1) * NT], ps)

            def proj_A(m):           # Q^T chunk m = Wq[:,mcols]^T x^T
                for t2 in range(T2):
                    ps = psmm.tile([P, NT], F32, tag="mm512")
                    for kc in range(KO):
                        nc.tensor.matmul(
                            ps, wq_sb[:, kc, _mcols(m)],
                            xT_sb[:, kc, t2 * NT:(t2 + 1) * NT],
                            start=(kc == 0), stop=(kc == KO - 1))
                    nc.vector.tensor_copy(
                        qt_sb[:, m, t2 * NT:(t2 + 1) * NT], ps)

            def proj_V():            # V = ctx Wv  (natural layout)
                for so in range(SO):
                    ps = psmm.tile([P, NT], F32, tag="mm512")
                    for kc in range(KO):
                        nc.tensor.matmul(
                            ps, ctxT_sb[:, kc, so * P:(so + 1) * P],
                            wv_sb[:, kc, :],
                            start=(kc == 0), stop=(kc == KO - 1))
                    nc.vector.tensor_copy(
                        vp_sb[:, so, :, 0:HD],
                        ps.rearrange("p (h d) -> p h d", h=HPG))

            # ---- attention unit: scores+exp+mask, then PV+normalize ----
            def scores_unit(h, t2):
                et = work.tile([P, SO, NT], BF16, tag="exp")
                for j in range(SO // 2):   # s-tile pairs share a 2-bank psum
                    ps = pssc.tile([P, 2 * NT], F32, tag="ps_sc")
                    for i in range(2):
                        so = 2 * j + i
                        nc.tensor.matmul(
                            ps[:, i * NT:(i + 1) * NT],
                            kt_sb[_hp(h), h // 2, so * P:(so + 1) * P],
                            qt_sb[_hp(h), h // 2, t2 * NT:(t2 + 1) * NT],
                            start=True, stop=True)
                    nc.scalar.activation(
                        et[:, 2 * j:2 * j + 2, :].rearrange("p a b -> p (a b)"),
                        ps, ACTF.Exp, scale=1.0 / 32.0)
                    nc.vector.tensor_tensor(
                        et[:, 2 * j:2 * j + 2, :],
                        et[:, 2 * j:2 * j + 2, :],
                        mask_sb[:, 2 * j:2 * j + 2, t2 * NT:(t2 + 1) * NT],
                        ALU.mult)
                return et

            def pv_unit(h, t2, et):
                psu = psu_pool.tile([HD + 1, NT], F32, tag="ps_u")
                for so in range(SO):
                    nc.tensor.matmul(
                        psu, vp_sb[:, so, h, :], et[:, so, :],
                        start=(so == 0), stop=(so == SO - 1))
                rc = work.tile([1, NT], F32, tag="recip")
                nc.vector.reciprocal(rc, psu[HD:HD + 1, :])
                bc = work.tile([HD, NT], F32, tag="bcast")
                nc.gpsimd.partition_broadcast(bc, rc)
                nc.vector.tensor_tensor(
                    ot_sb[_hp(h), h // 2, t2 * NT:(t2 + 1) * NT],
                    psu[0:HD, :], bc, ALU.mult)

            TM_HALF = NT // P        # 4 t-tiles per t2 half

            def stage_D(t2):         # out rows [t2*NT, (t2+1)*NT)
                for tm in range(t2 * TM_HALF, (t2 + 1) * TM_HALF):
                    for c2 in range(C // NT):
                        ps = psmm.tile([P, NT], F32, tag="mm512")
                        for kp in range(KP):
                            nc.tensor.matmul(
                                ps, ot_sb[:, kp, tm * P:(tm + 1) * P],
                                wo_sb[:, kp, c2 * NT:(c2 + 1) * NT],
                                start=(kp == 0), stop=(kp == KP - 1))
                        o_sb = work.tile([P, NT], F32, tag="out")
                        nc.vector.tensor_copy(o_sb, ps)
                        nc.sync.dma_start(
                            out[tm * P:(tm + 1) * P, c2 * NT:(c2 + 1) * NT],
                            o_sb)

            # ---- pipelined schedule ----
            # t-major unit order; scores run 2 units ahead of PV; remaining
            # projections and stage D(t2=0) are injected under the exp chain.
            units = [(h, t2) for t2 in range(T2) for h in range(HPG)]
            # pre[u]: issued BEFORE scores_unit u (data it depends on);
            # post[u]: issued right after (fills PE under the exp chain).
            pre = {2: [lambda: proj_B(1), lambda: proj_A(1)],
                   4: [lambda: proj_B(2), lambda: proj_A(2)],
                   6: [lambda: proj_B(3), lambda: proj_A(3)]}
            post = {0: [proj_V]}

            proj_B(0)
            proj_A(0)
            pending = []
            for u in range(len(units)):
                for f in pre.get(u, []):
                    f()
                pending.append(scores_unit(*units[u]))
                for f in post.get(u, []):
                    f()
                if u >= 2:
                    h, t2 = units[u - 2]
                    pv_unit(h, t2, pending.pop(0))
                if u == 11:          # P(7,0) just retired -> t half 0 done
                    stage_D(0)
            for u in range(len(units) - 2, len(units)):
                h, t2 = units[u]
                pv_unit(h, t2, pending.pop(0))
            stage_D(1)
    nc.compile()
    return nc


def _get_program():
    global _CACHED_NC
    if _CACHED_NC is None:
        _CACHED_NC = _build_program()
    return _CACHED_NC


def kernel(x, context, attn_mask, Wq, Wk, Wv, Wo):
    x = np.asarray(x, dtype=np.float32)
    context = np.asarray(context, dtype=np.float32)
    attn_mask = np.asarray(attn_mask)
    Wq = np.asarray(Wq, dtype=np.float32)
    Wk = np.asarray(Wk, dtype=np.float32)
    Wv = np.asarray(Wv, dtype=np.float32)
    Wo = np.asarray(Wo, dtype=np.float32)

    nc = _get_program()
    bf = ml_dtypes.bfloat16
    in_maps = []
    for i in range(NCORES):
        b, g = i // 2, i % 2
        cs = slice(g * CG, (g + 1) * CG)
        in_maps.append({
            "xT": np.ascontiguousarray(x[b].T).astype(bf),
            "ctxT": np.ascontiguousarray(context[b].T).astype(bf),
            "maskT": np.ascontiguousarray(attn_mask[b, 0].T).astype(bf),
            "wq": np.ascontiguousarray(Wq[:, cs]).astype(bf),
            "wk": np.ascontiguousarray(Wk[:, cs]).astype(bf),
            "wv": np.ascontiguousarray(Wv[:, cs]).astype(bf),
            "wo": np.ascontiguousarray(Wo[cs, :]),
        })

    profile = os.environ.get("KERNEL_PROFILE", "0") == "1"
    if profile:
        _ensure_ntff_hook()
    res = run_bass_kernel_spmd(
        nc, in_maps, list(range(NCORES)),
        trace=profile, trace_cores=[0] if profile else None)
    if profile:
        kernel.last_exec_time_ns = res.exec_time_ns
        kernel.last_trace = res.instructions_and_trace

    out = np.empty((B, T, C), dtype=np.float32)
    for b in range(B):
        out[b] = res.results[2 * b]["out"] + res.results[2 * b + 1]["out"]
    return out


# revision 7
# speedup vs baseline: 1.4727x; 1.4727x over previous
"""Cross-attention layer on 8 TRN2 NeuronCores.

Sharding: core i -> (batch b = i//2, head-group g = i%2); each core computes
its head-group's contribution to out[b] through Wo; the host sums the two
partial products per batch (row-split of Wo => partial-sum reduction).

Device kernel works in transposed layout ([channels, tokens]) so the softmax
reduction is along the matmul free axis:
  Q^T = Wq_g^T x^T, K^T = Wk_g^T ctx^T, V = ctx Wv_g (+ ones column)
  scores^T_h = K_h Q_h^T  (contraction over head_dim=64)
  E = exp(scores^T/32) * mask^T      (no max subtraction; |scores/32| ~ 1.5)
  U = V'^T E  (per s-tile accumulation; row 64 = softmax denominator)
  O^T = U[0:64] * recip(U[64])       (DVE reciprocal_approx + gpsimd bcast)
  out_partial = O^T^T Wo_g           (host adds core pairs)

The ACT exp chain is the stage-C pacer (~4.6us/unit, 16 units).  Issue
order is software-pipelined around it: attention units start right after
the first K/Q column chunks are projected; the remaining projections, the
V projection (split in halves) and the first half of the out-projection
run on the PE underneath the exp chain.  All DMAs are contiguous
kc-chunks issued in consumption order so no engine ever waits long.

Dtype split: x/ctx/Wq/Wk/Wv, Q^T/K^T/V', probs run in bf16 (pre-softmax
noise is negligible, probs/V noise ~0.5%); U accumulates in fp32 PSUM; the
normalized O^T and the Wo projection run in float32r (~1e-4).
"""

import os
import numpy as np
import ml_dtypes

import concourse.mybir as mybir
from concourse import bacc
import concourse.tile as tile
from concourse.bass_utils import run_bass_kernel_spmd

B, T, TC = 4, 1024, 1024
C, CTX_C, H = 1024, 1024, 16
HD = C // H            # 64
P = 128
NCORES = 8
HG = 2                 # head groups
HPG = H // HG          # 8 heads per core
CG = HPG * HD          # 512 channels per group
NT = 512               # matmul free-dim chunk
KO = C // P            # 8 contraction tiles for projections
MQ = CG // P           # 4 partition-tiles of Q^T/K^T
SO = TC // P           # 8 s-tiles
T2 = T // NT           # 2 t-chunks
KP = CG // P           # 4 contraction tiles for the out projection
TM_HALF = NT // P      # 4 t-tiles of stage D per t2 half
F32 = mybir.dt.float32
F32R = mybir.dt.float32r
BF16 = mybir.dt.bfloat16
ALU = mybir.AluOpType
ACTF = mybir.ActivationFunctionType

_CACHED_NC = None


def _ensure_ntff_hook():
    """Register the axon NTFF profiling hook if the image's antenv lacks it."""
    try:
        from antenv.axon_hooks import get_axon_ntff_profile_hook  # noqa: F401
        return
    except ImportError:
        pass
    import sys
    import types
    try:
        from trn_agent_boot.trn_boot import _ntff_profile_via_ctypes
        hook = _ntff_profile_via_ctypes("/opt/axon/libaxon_pjrt.so")
    except Exception:
        hook = None
    mod = types.ModuleType("antenv.axon_hooks")
    mod.get_axon_ntff_profile_hook = lambda: hook
    mod.set_axon_ntff_profile_hook = lambda h: None
    sys.modules["antenv.axon_hooks"] = mod
    import antenv
    antenv.axon_hooks = mod


def _hp(h):
    """Partition slice of local head h inside a [128, MQ, ...] channel tile."""
    lo = (h % 2) * HD
    return slice(lo, lo + HD)


def _build_program():
    nc = bacc.Bacc("TRN2", target_bir_lowering=False, debug=False,
                   num_devices=NCORES)
    xT = nc.dram_tensor("xT", [C, T], BF16, kind="ExternalInput").ap()
    ctxT = nc.dram_tensor("ctxT", [CTX_C, TC], BF16, kind="ExternalInput").ap()
    maskT = nc.dram_tensor("maskT", [TC, T], BF16, kind="ExternalInput").ap()
    wq = nc.dram_tensor("wq", [C, CG], BF16, kind="ExternalInput").ap()
    wk = nc.dram_tensor("wk", [CTX_C, CG], BF16, kind="ExternalInput").ap()
    wv = nc.dram_tensor("wv", [CTX_C, CG], BF16, kind="ExternalInput").ap()
    wo = nc.dram_tensor("wo", [CG, C], F32, kind="ExternalInput").ap()
    out = nc.dram_tensor("out", [T, C], F32, kind="ExternalOutput").ap()

    with tile.TileContext(nc) as tc:
        with (
            tc.tile_pool(name="persist", bufs=1) as persist,
            tc.tile_pool(name="work", bufs=3) as work,
            tc.tile_pool(name="psmm", bufs=2, space="PSUM") as psmm,
            tc.tile_pool(name="pssc", bufs=2, space="PSUM") as pssc,
            tc.tile_pool(name="psu", bufs=2, space="PSUM") as psu_pool,
        ):
            qt_sb = persist.tile([P, MQ, T], BF16)            # Q^T [(h,d), t]
            kt_sb = persist.tile([P, MQ, TC], BF16)           # K^T [(h,d), s]
            vp_sb = persist.tile([P, SO, HPG, HD + 1], BF16)  # V' + ones col
            mask_sb = persist.tile([P, SO, T], BF16)          # mask^T
            ot_sb = persist.tile([P, KP, T], F32R)            # O^T normalized
            wo_sb = persist.tile([P, KP, C], F32R)
            xT_sb = persist.tile([P, KO, T], BF16)
            ctxT_sb = persist.tile([P, KO, TC], BF16)
            wq_sb = persist.tile([P, KO, CG], BF16)
            wk_sb = persist.tile([P, KO, CG], BF16)
            wv_sb = persist.tile([P, KO, CG], BF16)

            nc.gpsimd.memset(vp_sb[:, :, :, HD:HD + 1], 1.0)

            xT_r = xT.rearrange("(ko p) t -> p ko t", p=P)
            ctxT_r = ctxT.rearrange("(ko p) t -> p ko t", p=P)
            wq_r = wq.rearrange("(ko p) m -> p ko m", p=P)
            wk_r = wk.rearrange("(ko p) m -> p ko m", p=P)
            wv_r = wv.rearrange("(ko p) m -> p ko m", p=P)
            wo_r = wo.rearrange("(ko p) n -> p ko n", p=P).bitcast(F32R)
            mask_r = maskT.rearrange("(so p) t -> p so t", p=P)

            # DMAs: contiguous kc-chunks, in consumption order.
            for kc in range(KO):
                nc.sync.dma_start(wk_sb[:, kc], wk_r[:, kc])
            for kc in range(KO):
                nc.sync.dma_start(ctxT_sb[:, kc], ctxT_r[:, kc])
            for kc in range(KO):
                nc.sync.dma_start(wq_sb[:, kc], wq_r[:, kc])
            for kc in range(KO):
                nc.sync.dma_start(xT_sb[:, kc, 0:NT], xT_r[:, kc, 0:NT])
            nc.sync.dma_start(mask_sb[:, 0:2], mask_r[:, 0:2])
            nc.sync.dma_start(mask_sb[:, 2:4], mask_r[:, 2:4])
            for kc in range(KO):
                nc.sync.dma_start(wv_sb[:, kc], wv_r[:, kc])
            nc.sync.dma_start(mask_sb[:, 4:6], mask_r[:, 4:6])
            nc.sync.dma_start(mask_sb[:, 6:8], mask_r[:, 6:8])
            for kc in range(KO):
                nc.sync.dma_start(xT_sb[:, kc, NT:T], xT_r[:, kc, NT:T])
            for kc in range(KO):
                nc.sync.dma_start(wo_sb[:, kc // 2, (kc % 2) * NT:
                                        (kc % 2) * NT + NT],
                                  wo_r[:, kc // 2, (kc % 2) * NT:
                                       (kc % 2) * NT + NT])

            def _mcols(m):
                return slice(m * P, (m + 1) * P)

            # ---- projection building blocks ----
            def proj_B(m):           # K^T chunk m = Wk[:,mcols]^T ctx^T
                for s2 in range(T2):
                    ps = psmm.tile([P, NT], F32, tag="mm512")
                    for kc in range(KO):
                        nc.tensor.matmul(
                            ps, wk_sb[:, kc, _mcols(m)],
                            ctxT_sb[:, kc, s2 * NT:(s2 + 1) * NT],
                            start=(kc == 0), stop=(kc == KO - 1))
                    nc.vector.tensor_copy(
                        kt_sb[:, m, s2 * NT:(s2 + 1) * NT], ps)

            def proj_A(m, t2):       # Q^T chunk m, t half t2
                ps = psmm.tile([P, NT], F32, tag="mm512")
                for kc in range(KO):
                    nc.tensor.matmul(
                        ps, wq_sb[:, kc, _mcols(m)],
                        xT_sb[:, kc, t2 * NT:(t2 + 1) * NT],
                        start=(kc == 0), stop=(kc == KO - 1))
                nc.vector.tensor_copy(
                    qt_sb[:, m, t2 * NT:(t2 + 1) * NT], ps)

            def proj_V(half):        # V = ctx Wv  (natural layout), 4 s-tiles
                for so in range(half * 4, half * 4 + 4):
                    ps = psmm.tile([P, NT], F32, tag="mm512")
                    for kc in range(KO):
                        nc.tensor.matmul(
                            ps, ctxT_sb[:, kc, so * P:(so + 1) * P],
                            wv_sb[:, kc, :],
                            start=(kc == 0), stop=(kc == KO - 1))
                    nc.vector.tensor_copy(
                        vp_sb[:, so, :, 0:HD],
                        ps.rearrange("p (h d) -> p h d", h=HPG))

            # ---- attention unit: scores+exp+mask, then PV+normalize ----
            def scores_unit(h, t2):
                et = work.tile([P, SO, NT], BF16, tag="exp")
                for j in range(SO // 2):   # s-tile pairs share a 2-bank psum
                    ps = pssc.tile([P, 2 * NT], F32, tag="ps_sc")
                    for i in range(2):
                        so = 2 * j + i
                        nc.tensor.matmul(
                            ps[:, i * NT:(i + 1) * NT],
                            kt_sb[_hp(h), h // 2, so * P:(so + 1) * P],
                            qt_sb[_hp(h), h // 2, t2 * NT:(t2 + 1) * NT],
                            start=True, stop=True)
                    nc.scalar.activation(
                        et[:, 2 * j:2 * j + 2, :].rearrange("p a b -> p (a b)"),
                        ps, ACTF.Exp, scale=1.0 / 32.0)
                    nc.vector.tensor_tensor(
                        et[:, 2 * j:2 * j + 2, :],
                        et[:, 2 * j:2 * j + 2, :],
                        mask_sb[:, 2 * j:2 * j + 2, t2 * NT:(t2 + 1) * NT],
                        ALU.mult)
                return et

            def pv_unit(h, t2, et):
                psu = psu_pool.tile([HD + 1, NT], F32, tag="ps_u")
                for so in range(SO):
                    nc.tensor.matmul(
                        psu, vp_sb[:, so, h, :], et[:, so, :],
                        start=(so == 0), stop=(so == SO - 1))
                den = work.tile([1, NT], F32, tag="den")
                nc.vector.tensor_copy(den, psu[HD:HD + 1, :])
                rc = work.tile([1, NT], F32, tag="recip")
                nc.vector.reciprocal_approx_fast(out=rc, in_=den)
                bc = work.tile([HD, NT], F32, tag="bcast")
                nc.gpsimd.partition_broadcast(bc, rc)
                nc.vector.tensor_tensor(
                    ot_sb[_hp(h), h // 2, t2 * NT:(t2 + 1) * NT],
                    psu[0:HD, :], bc, ALU.mult)

            def stage_D(tm_lo, tm_hi, copies_on_act):
                for tm in range(tm_lo, tm_hi):
                    for c2 in range(C // NT):
                        ps = psmm.tile([P, NT], F32, tag="mm512")
                        for kp in range(KP):
                            nc.tensor.matmul(
                                ps, ot_sb[:, kp, tm * P:(tm + 1) * P],
                                wo_sb[:, kp, c2 * NT:(c2 + 1) * NT],
                                start=(kp == 0), stop=(kp == KP - 1))
                        o_sb = work.tile([P, NT], F32, tag="out")
                        if copies_on_act:
                            nc.scalar.activation(o_sb, ps, ACTF.Copy)
                        else:
                            nc.vector.tensor_copy(o_sb, ps)
                        nc.sync.dma_start(
                            out[tm * P:(tm + 1) * P, c2 * NT:(c2 + 1) * NT],
                            o_sb)

            # ---- pipelined schedule ----
            # t-major unit order; scores run 2 units ahead of PV; projections
            # and the first out-projection half fill the PE under the exp
            # chain (each filler block <= ~7us so ACT never starves).
            units = [(h, t2) for t2 in range(T2) for h in range(HPG)]
            pre = {2: [lambda: proj_B(1), lambda: proj_A(1, 0)],
                   4: [lambda: proj_B(2), lambda: proj_A(2, 0)],
                   6: [lambda: proj_B(3), lambda: proj_A(3, 0)],
                   7: [lambda: proj_A(0, 1), lambda: proj_A(1, 1)],
                   10: [lambda: proj_A(2, 1)],
                   12: [lambda: proj_A(3, 1)]}
            post = {0: [lambda: proj_V(0)],
                    1: [lambda: proj_V(1)],
                    11: [lambda: stage_D(0, 2, False)],
                    12: [lambda: stage_D(2, 4, False)]}

            proj_B(0)
            proj_A(0, 0)
            pending = []
            for u in range(len(units)):
                for f in pre.get(u, []):
                    f()
                pending.append(scores_unit(*units[u]))
                for f in post.get(u, []):
                    f()
                if u >= 2:
                    h, t2 = units[u - 2]
                    pv_unit(h, t2, pending.pop(0))
            for u in range(len(units) - 2, len(units)):
                h, t2 = units[u]
                pv_unit(h, t2, pending.pop(0))
            stage_D(TM_HALF, 2 * TM_HALF, True)
    nc.compile()
    return nc


def _get_program():
    global _CACHED_NC
    if _CACHED_NC is None:
        _CACHED_NC = _build_program()
    return _CACHED_NC


def kernel(x, context, attn_mask, Wq, Wk, Wv, Wo):
    x = np.asarray(x, dtype=np.float32)
    context = np.asarray(context, dtype=np.float32)
    attn_mask = np.asarray(attn_mask)
    Wq = np.asarray(Wq, dtype=np.float32)
    Wk = np.asarray(Wk, dtype=np.float32)
    Wv = np.asarray(Wv, dtype=np.float32)
    Wo = np.asarray(Wo, dtype=np.float32)

    nc = _get_program()
    bf = ml_dtypes.bfloat16
    in_maps = []
    for i in range(NCORES):
        b, g = i // 2, i % 2
        cs = slice(g * CG, (g + 1) * CG)
        in_maps.append({
            "xT": np.ascontiguousarray(x[b].T).astype(bf),
            "ctxT": np.ascontiguousarray(context[b].T).astype(bf),
            "maskT": np.ascontiguousarray(attn_mask[b, 0].T).astype(bf),
            "wq": np.ascontiguousarray(Wq[:, cs]).astype(bf),
            "wk": np.ascontiguousarray(Wk[:, cs]).astype(bf),
            "wv": np.ascontiguousarray(Wv[:, cs]).astype(bf),
            "wo": np.ascontiguousarray(Wo[cs, :]),
        })

    profile = os.environ.get("KERNEL_PROFILE", "0") == "1"
    if profile:
        _ensure_ntff_hook()
    res = run_bass_kernel_spmd(
        nc, in_maps, list(range(NCORES)),
        trace=profile, trace_cores=[0] if profile else None)
    if profile:
        kernel.last_exec_time_ns = res.exec_time_ns
        kernel.last_trace = res.instructions_and_trace

    out = np.empty((B, T, C), dtype=np.float32)
    for b in range(B):
        out[b] = res.results[2 * b]["out"] + res.results[2 * b + 1]["out"]
    return out


# revision 10
# speedup vs baseline: 1.4980x; 1.0172x over previous
"""Cross-attention layer on 8 TRN2 NeuronCores.

Sharding: core i -> (batch b = i//2, head-group g = i%2); each core computes
its head-group's contribution to out[b] through Wo; the host sums the two
partial products per batch (row-split of Wo => partial-sum reduction).

Device kernel works in transposed layout ([channels, tokens]) so the softmax
reduction is along the matmul free axis:
  Q^T = Wq_g^T x^T, K^T = Wk_g^T ctx^T, V' = [1 | ctx Wv_g]  (ones col 0)
  scores^T_h = K_h Q_h^T  (contraction over head_dim=64)
  E = exp(scores^T/32) * mask^T      (no max subtraction; |scores/32| ~ 1.5)
  U = V'^T E  (per s-tile accumulation; row 0 = softmax denominator)
  O^T = U[1:65] * recip(U[0])        (DVE reciprocal_approx + gpsimd bcast)
  out_partial = O^T^T Wo_g           (host adds core pairs)

The ACT exp chain is the stage-C pacer (~4.5us/unit, 16 units).  Issue
order (= scheduler priority) is software-pipelined around it: scores units
run 3 units ahead of PV; the projections, the V projection and the first
half of the out-projection fill the PE underneath the exp chain.  Wq/Wk
arrive host-pre-chunked per column block so the first attention unit's
inputs need only ~3.5MB of DMA before the exp chain starts.

Dtype split: x/ctx/Wq/Wk/Wv, Q^T/K^T/V', probs run in bf16 (pre-softmax
noise is negligible, probs/V noise ~0.5%); U accumulates in fp32 PSUM; the
normalized O^T and the Wo projection run in float32r (~1e-4).
"""

import os
import numpy as np
import ml_dtypes

import concourse.mybir as mybir
from concourse import bacc
import concourse.tile as tile
from concourse.bass_utils import run_bass_kernel_spmd

B, T, TC = 4, 1024, 1024
C, CTX_C, H = 1024, 1024, 16
HD = C // H            # 64
P = 128
NCORES = 8
HG = 2                 # head groups
HPG = H // HG          # 8 heads per core
CG = HPG * HD          # 512 channels per group
NT = 512               # matmul free-dim chunk
KO = C // P            # 8 contraction tiles for projections
MQ = CG // P           # 4 partition-tiles of Q^T/K^T
SO = TC // P           # 8 s-tiles
T2 = T // NT           # 2 t-chunks
KP = CG // P           # 4 contraction tiles for the out projection
TM_HALF = NT // P      # 4 t-tiles of stage D per t2 half
F32 = mybir.dt.float32
F32R = mybir.dt.float32r
BF16 = mybir.dt.bfloat16
ALU = mybir.AluOpType
ACTF = mybir.ActivationFunctionType

_CACHED_NC = None


def _ensure_ntff_hook():
    """Register the axon NTFF profiling hook if the image's antenv lacks it."""
    try:
        from antenv.axon_hooks import get_axon_ntff_profile_hook  # noqa: F401
        return
    except ImportError:
        pass
    import sys
    import types
    try:
        from trn_agent_boot.trn_boot import _ntff_profile_via_ctypes
        hook = _ntff_profile_via_ctypes("/opt/axon/libaxon_pjrt.so")
    except Exception:
        hook = None
    mod = types.ModuleType("antenv.axon_hooks")
    mod.get_axon_ntff_profile_hook = lambda: hook
    mod.set_axon_ntff_profile_hook = lambda h: None
    sys.modules["antenv.axon_hooks"] = mod
    import antenv
    antenv.axon_hooks = mod


def _hp(h):
    """Partition slice of local head h inside a [128, MQ, ...] channel tile."""
    lo = (h % 2) * HD
    return slice(lo, lo + HD)


def _build_program():
    nc = bacc.Bacc("TRN2", target_bir_lowering=False, debug=False,
                   num_devices=NCORES)
    xT = nc.dram_tensor("xT", [C, T], BF16, kind="ExternalInput").ap()
    ctxT = nc.dram_tensor("ctxT", [CTX_C, TC], BF16, kind="ExternalInput").ap()
    maskT = nc.dram_tensor("maskT", [TC, T], BF16, kind="ExternalInput").ap()
    # wq/wk host-pre-chunked: [MQ][P][KO][P] so one m-chunk is a single
    # contiguous-per-partition DMA (2KB lines).
    wqm = nc.dram_tensor("wqm", [MQ, P, KO, P], BF16, kind="ExternalInput").ap()
    wkm = nc.dram_tensor("wkm", [MQ, P, KO, P], BF16, kind="ExternalInput").ap()
    wv = nc.dram_tensor("wv", [CTX_C, CG], BF16, kind="ExternalInput").ap()
    wo = nc.dram_tensor("wo", [CG, C], F32, kind="ExternalInput").ap()
    out = nc.dram_tensor("out", [T, C], F32, kind="ExternalOutput").ap()

    with tile.TileContext(nc) as tc:
        with (
            tc.tile_pool(name="persist", bufs=1) as persist,
            tc.tile_pool(name="etp", bufs=4) as etp,
            tc.tile_pool(name="work", bufs=3) as work,
            tc.tile_pool(name="psmm", bufs=2, space="PSUM") as psmm,
            tc.tile_pool(name="pssc", bufs=2, space="PSUM") as pssc,
            tc.tile_pool(name="psu", bufs=2, space="PSUM") as psu_pool,
        ):
            qt_sb = persist.tile([P, MQ, T], BF16)            # Q^T [(h,d), t]
            kt_sb = persist.tile([P, MQ, TC], BF16)           # K^T [(h,d), s]
            vp_sb = persist.tile([P, SO, HPG, P], BF16)       # [1|pad63|V64]
            mask_sb = persist.tile([P, SO, T], BF16)          # mask^T
            ot_sb = persist.tile([P, KP, T], F32R)            # O^T normalized
            wo_sb = persist.tile([P, KP, C], F32R)
            xT_sb = persist.tile([P, KO, T], BF16)
            ctxT_sb = persist.tile([P, KO, TC], BF16)
            wq_sb = persist.tile([P, KO, CG], BF16)
            wk_sb = persist.tile([P, KO, CG], BF16)
            wv_sb = persist.tile([P, KO, CG], BF16)

            nc.gpsimd.memset(vp_sb[:, :, :, 0:1], 1.0)

            xT_r = xT.rearrange("(ko p) t -> p ko t", p=P)
            ctxT_r = ctxT.rearrange("(ko p) t -> p ko t", p=P)
            wv_r = wv.rearrange("(ko p) m -> p ko m", p=P)
            wo_r = wo.rearrange("(ko p) n -> p ko n", p=P).bitcast(F32R)
            mask_r = maskT.rearrange("(so p) t -> p so t", p=P)

            def _mcols(m):
                return slice(m * P, (m + 1) * P)

            # DMAs: contiguous chunks, in consumption order.
            nc.sync.dma_start(wk_sb[:, :, 0:P], wkm[0])
            for half in range(2):
                sl = slice(half * NT, half * NT + NT)
                for kc in range(KO):
                    nc.sync.dma_start(ctxT_sb[:, kc, sl], ctxT_r[:, kc, sl])
            nc.sync.dma_start(wq_sb[:, :, 0:P], wqm[0])
            for kc in range(KO):
                nc.sync.dma_start(xT_sb[:, kc, 0:NT], xT_r[:, kc, 0:NT])
            for j in range(SO // 2):
                nc.sync.dma_start(mask_sb[:, 2 * j:2 * j + 2],
                                  mask_r[:, 2 * j:2 * j + 2])
            for kc in range(KO):
                nc.sync.dma_start(wv_sb[:, kc], wv_r[:, kc])
            nc.sync.dma_start(wk_sb[:, :, _mcols(1)], wkm[1])
            nc.sync.dma_start(wq_sb[:, :, _mcols(1)], wqm[1])
            for kc in range(KO):
                nc.sync.dma_start(xT_sb[:, kc, NT:T], xT_r[:, kc, NT:T])
            for m in range(2, MQ):
                nc.sync.dma_start(wk_sb[:, :, _mcols(m)], wkm[m])
                nc.sync.dma_start(wq_sb[:, :, _mcols(m)], wqm[m])
            for kc in range(KO):
                nc.sync.dma_start(wo_sb[:, kc // 2, (kc % 2) * NT:
                                        (kc % 2) * NT + NT],
                                  wo_r[:, kc // 2, (kc % 2) * NT:
                                       (kc % 2) * NT + NT])

            # ---- projection building blocks ----
            def proj_B(m, casts_on_act=False):
                for s2 in range(T2):
                    ps = psmm.tile([P, NT], F32, tag="mm512")
                    for kc in range(KO):
                        nc.tensor.matmul(
                            ps, wk_sb[:, kc, _mcols(m)],
                            ctxT_sb[:, kc, s2 * NT:(s2 + 1) * NT],
                            start=(kc == 0), stop=(kc == KO - 1))
                    dst = kt_sb[:, m, s2 * NT:(s2 + 1) * NT]
                    if casts_on_act:
                        nc.scalar.activation(dst, ps, ACTF.Copy)
                    else:
                        nc.vector.tensor_copy(dst, ps)

            def proj_A(m, t2, casts_on_act=False):
                ps = psmm.tile([P, NT], F32, tag="mm512")
                for kc in range(KO):
                    nc.tensor.matmul(
                        ps, wq_sb[:, kc, _mcols(m)],
                        xT_sb[:, kc, t2 * NT:(t2 + 1) * NT],
                        start=(kc == 0), stop=(kc == KO - 1))
                dst = qt_sb[:, m, t2 * NT:(t2 + 1) * NT]
                if casts_on_act:
                    nc.scalar.activation(dst, ps, ACTF.Copy)
                else:
                    nc.vector.tensor_copy(dst, ps)

            def proj_V(half):        # V = ctx Wv  (natural layout), 4 s-tiles
                for so in range(half * 4, half * 4 + 4):
                    ps = psmm.tile([P, NT], F32, tag="mm512")
                    for kc in range(KO):
                        nc.tensor.matmul(
                            ps, ctxT_sb[:, kc, so * P:(so + 1) * P],
                            wv_sb[:, kc, :],
                            start=(kc == 0), stop=(kc == KO - 1))
                    nc.vector.tensor_copy(
                        vp_sb[:, so, :, 64:64 + HD],
                        ps.rearrange("p (h d) -> p h d", h=HPG))

            # ---- attention unit: scores+exp+mask, then PV+normalize ----
            def scores_unit(h, t2):
                et = etp.tile([P, SO, NT], BF16, tag="exp")
                for j in range(SO // 2):   # s-tile pairs share a 2-bank psum
                    ps = pssc.tile([P, 2 * NT], F32, tag="ps_sc")
                    for i in range(2):
                        so = 2 * j + i
                        nc.tensor.matmul(
                            ps[:, i * NT:(i + 1) * NT],
                            kt_sb[_hp(h), h // 2, so * P:(so + 1) * P],
                            qt_sb[_hp(h), h // 2, t2 * NT:(t2 + 1) * NT],
                            start=True, stop=True)
                    nc.scalar.activation(
                        et[:, 2 * j:2 * j + 2, :].rearrange("p a b -> p (a b)"),
                        ps, ACTF.Exp, scale=1.0 / 32.0)
                    nc.vector.tensor_tensor(
                        et[:, 2 * j:2 * j + 2, :],
                        et[:, 2 * j:2 * j + 2, :],
                        mask_sb[:, 2 * j:2 * j + 2, t2 * NT:(t2 + 1) * NT],
                        ALU.mult)
                return et

            def pv_unit(h, t2, et):
                psu = psu_pool.tile([P, NT], F32, tag="ps_u")
                for so in range(SO):
                    nc.tensor.matmul(
                        psu, vp_sb[:, so, h, :], et[:, so, :],
                        start=(so == 0), stop=(so == SO - 1))
                rc = work.tile([1, NT], F32, tag="recip")
                nc.vector.reciprocal_approx_fast(out=rc, in_=psu[0:1, :])
                bc = work.tile([HD, NT], F32, tag="bcast")
                nc.gpsimd.partition_broadcast(bc, rc)
                nc.vector.tensor_tensor(
                    ot_sb[_hp(h), h // 2, t2 * NT:(t2 + 1) * NT],
                    psu[64:64 + HD, :], bc, ALU.mult)

            def stage_D(tm_lo, tm_hi, copies_on_act):
                for tm in range(tm_lo, tm_hi):
                    for c2 in range(C // NT):
                        ps = psmm.tile([P, NT], F32, tag="mm512")
                        for kp in range(KP):
                            nc.tensor.matmul(
                                ps, ot_sb[:, kp, tm * P:(tm + 1) * P],
                                wo_sb[:, kp, c2 * NT:(c2 + 1) * NT],
                                start=(kp == 0), stop=(kp == KP - 1))
                        o_sb = work.tile([P, NT], F32, tag="out")
                        if copies_on_act:
                            nc.scalar.activation(o_sb, ps, ACTF.Copy)
                        else:
                            nc.vector.tensor_copy(o_sb, ps)
                        nc.sync.dma_start(
                            out[tm * P:(tm + 1) * P, c2 * NT:(c2 + 1) * NT],
                            o_sb)

            # ---- pipelined schedule (issue order = scheduler priority) ----
            ets = {}

            def S(h, t2):
                ets[(h, t2)] = scores_unit(h, t2)

            def PV(h, t2):
                pv_unit(h, t2, ets.pop((h, t2)))

            proj_B(0, casts_on_act=True)
            proj_A(0, 0, casts_on_act=True)
            S(0, 0)
            S(1, 0)
            proj_B(1, casts_on_act=True)
            proj_A(1, 0, casts_on_act=True)
            S(2, 0)
            proj_V(0)
            proj_V(1)
            S(3, 0); PV(0, 0)
            proj_B(2)
            proj_A(2, 0)
            S(4, 0); PV(1, 0)
            proj_B(3)
            proj_A(3, 0)
            S(5, 0); PV(2, 0)
            S(6, 0); PV(3, 0)
            S(7, 0); PV(4, 0)
            proj_A(0, 1)
            proj_A(1, 1)
            S(0, 1); PV(5, 0)
            S(1, 1); PV(6, 0)
            proj_A(2, 1)
            proj_A(3, 1)
            S(2, 1); PV(7, 0)
            stage_D(0, 2, False)
            S(3, 1); PV(0, 1)
            stage_D(2, 4, False)
            S(4, 1); PV(1, 1)
            S(5, 1); PV(2, 1)
            S(6, 1); PV(3, 1)
            S(7, 1); PV(4, 1)
            PV(5, 1)
            PV(6, 1)
            PV(7, 1)
            stage_D(TM_HALF, 2 * TM_HALF, True)
    nc.compile()
    return nc


def _get_program():
    global _CACHED_NC
    if _CACHED_NC is None:
        _CACHED_NC = _build_program()
    return _CACHED_NC


def kernel(x, context, attn_mask, Wq, Wk, Wv, Wo):
    x = np.asarray(x, dtype=np.float32)
    context = np.asarray(context, dtype=np.float32)
    attn_mask = np.asarray(attn_mask)
    Wq = np.asarray(Wq, dtype=np.float32)
    Wk = np.asarray(Wk, dtype=np.float32)
    Wv = np.asarray(Wv, dtype=np.float32)
    Wo = np.asarray(Wo, dtype=np.float32)

    nc = _get_program()
    bf = ml_dtypes.bfloat16

    def _mchunk(w):
        # [C, CG_slice] -> [MQ, P, KO, P]: per column-block, partition-major
        return np.ascontiguousarray(
            w.reshape(KO, P, MQ, P).transpose(2, 1, 0, 3)).astype(bf)

    in_maps = []
    for i in range(NCORES):
        b, g = i // 2, i % 2
        cs = slice(g * CG, (g + 1) * CG)
        in_maps.append({
            "xT": np.ascontiguousarray(x[b].T).astype(bf),
            "ctxT": np.ascontiguousarray(context[b].T).astype(bf),
            "maskT": np.ascontiguousarray(attn_mask[b, 0].T).astype(bf),
            "wqm": _mchunk(Wq[:, cs]),
            "wkm": _mchunk(Wk[:, cs]),
            "wv": np.ascontiguousarray(Wv[:, cs]).astype(bf),
            "wo": np.ascontiguousarray(Wo[cs, :]),
        })

    profile = os.environ.get("KERNEL_PROFILE", "0") == "1"
    if profile:
        _ensure_ntff_hook()
    res = run_bass_kernel_spmd(
        nc, in_maps, list(range(NCORES)),
        trace=profile, trace_cores=[0] if profile else None)
    if profile:
        kernel.last_exec_time_ns = res.exec_time_ns
        kernel.last_trace = res.instructions_and_trace

    out = np.empty((B, T, C), dtype=np.float32)
    for b in range(B):
        out[b] = res.results[2 * b]["out"] + res.results[2 * b + 1]["out"]
    return out


# revision 13
# speedup vs baseline: 1.5367x; 1.0259x over previous
"""Cross-attention layer on 8 TRN2 NeuronCores.

Sharding: core i -> (batch b = i//2, head-group g = i%2); each core computes
its head-group's contribution to out[b] through Wo; the host sums the two
partial products per batch (row-split of Wo => partial-sum reduction).

Device kernel works in transposed layout ([channels, tokens]) so the softmax
reduction is along the matmul free axis:
  Q^T = Wq_g^T x^T, K^T = Wk_g^T ctx^T, V' = [1 | pad | ctx Wv_g]
  scores^T_h = K_h Q_h^T  (contraction over head_dim=64; head pairs run as
                           concurrent PE row-tiles via base partitions 0/64)
  E = exp(scores^T/32) * mask^T      (no max subtraction; |scores/32| ~ 1.5)
  U = V'^T E  (per s-tile accumulation; row 0 = softmax denominator)
  O^T = U[64:128] * recip(U[0])      (DVE reciprocal_approx + gpsimd bcast)
  out_partial = O^T^T Wo_g           (host adds core pairs)

The kernel is co-limited by the ACT exp chain (~71us) and the HBM input
stream (~9MB at the ~160GB/s/core effective rate under 8-core contention),
so every DMA is issued in exact consumption order and the schedule is
software-pipelined: scores pair-units run ~3 head-units ahead of PV
(6 et buffers), and the projections / V / first out-projection half fill
the PE underneath the exp chain.

Dtype split: x/ctx/Wq/Wk/Wv, Q^T/K^T/V', probs, O^T, Wo and the out store
run in bf16; PSUM accumulation is fp32 everywhere; the softmax reciprocal
is fp32 (DVE approx, ~51 ULP).  Host sums the core pairs in fp32.
"""

import os
import numpy as np
import ml_dtypes

import concourse.mybir as mybir
from concourse import bacc
import concourse.tile as tile
from concourse.bass_utils import run_bass_kernel_spmd

B, T, TC = 4, 1024, 1024
C, CTX_C, H = 1024, 1024, 16
HD = C // H            # 64
P = 128
NCORES = 8
HG = 2                 # head groups
HPG = H // HG          # 8 heads per core
CG = HPG * HD          # 512 channels per group
NT = 512               # matmul free-dim chunk
KO = C // P            # 8 contraction tiles for projections
MQ = CG // P           # 4 partition-tiles of Q^T/K^T
SO = TC // P           # 8 s-tiles
T2 = T // NT           # 2 t-chunks
KP = CG // P           # 4 contraction tiles for the out projection
TM_HALF = NT // P      # 4 t-tiles of stage D per t2 half
F32 = mybir.dt.float32
BF16 = mybir.dt.bfloat16
ALU = mybir.AluOpType
ACTF = mybir.ActivationFunctionType

_CACHED_NC = None


def _ensure_ntff_hook():
    """Register the axon NTFF profiling hook if the image's antenv lacks it."""
    try:
        from antenv.axon_hooks import get_axon_ntff_profile_hook  # noqa: F401
        return
    except ImportError:
        pass
    import sys
    import types
    try:
        from trn_agent_boot.trn_boot import _ntff_profile_via_ctypes
        hook = _ntff_profile_via_ctypes("/opt/axon/libaxon_pjrt.so")
    except Exception:
        hook = None
    mod = types.ModuleType("antenv.axon_hooks")
    mod.get_axon_ntff_profile_hook = lambda: hook
    mod.set_axon_ntff_profile_hook = lambda h: None
    sys.modules["antenv.axon_hooks"] = mod
    import antenv
    antenv.axon_hooks = mod


def _hp(h):
    """Partition slice of local head h inside a [128, MQ, ...] channel tile."""
    lo = (h % 2) * HD
    return slice(lo, lo + HD)


def _build_program():
    nc = bacc.Bacc("TRN2", target_bir_lowering=False, debug=False,
                   num_devices=NCORES)
    xT = nc.dram_tensor("xT", [C, T], BF16, kind="ExternalInput").ap()
    ctxT = nc.dram_tensor("ctxT", [CTX_C, TC], BF16, kind="ExternalInput").ap()
    maskT = nc.dram_tensor("maskT", [TC, T], BF16, kind="ExternalInput").ap()
    # wq/wk host-pre-chunked: [MQ][P][KO][P] so one m-chunk is a single
    # contiguous-per-partition DMA (2KB lines).
    wqm = nc.dram_tensor("wqm", [MQ, P, KO, P], BF16, kind="ExternalInput").ap()
    wkm = nc.dram_tensor("wkm", [MQ, P, KO, P], BF16, kind="ExternalInput").ap()
    wv = nc.dram_tensor("wv", [CTX_C, CG], BF16, kind="ExternalInput").ap()
    wo = nc.dram_tensor("wo", [CG, C], BF16, kind="ExternalInput").ap()
    out = nc.dram_tensor("out", [T, C], BF16, kind="ExternalOutput").ap()

    with tile.TileContext(nc) as tc:
        with (
            tc.tile_pool(name="persist", bufs=1) as persist,
            tc.tile_pool(name="etp", bufs=6) as etp,
            tc.tile_pool(name="work", bufs=3) as work,
            tc.tile_pool(name="psmm", bufs=2, space="PSUM") as psmm,
            tc.tile_pool(name="pssc", bufs=2, space="PSUM") as pssc,
            tc.tile_pool(name="psu", bufs=2, space="PSUM") as psu_pool,
        ):
            qt_sb = persist.tile([P, MQ, T], BF16)            # Q^T [(h,d), t]
            kt_sb = persist.tile([P, MQ, TC], BF16)           # K^T [(h,d), s]
            vp_sb = persist.tile([P, SO, HPG, P], BF16)       # [1|pad63|V64]
            mask_sb = persist.tile([P, SO, T], BF16)          # mask^T
            ot_sb = persist.tile([P, KP, T], BF16)            # O^T normalized
            wo_sb = persist.tile([P, KP, C], BF16)
            xT_sb = persist.tile([P, KO, T], BF16)
            ctxT_sb = persist.tile([P, KO, TC], BF16)
            wq_sb = persist.tile([P, KO, CG], BF16)
            wk_sb = persist.tile([P, KO, CG], BF16)
            wv_sb = persist.tile([P, KO, CG], BF16)

            nc.gpsimd.memset(vp_sb[:, :, :, 0:1], 1.0)

            xT_r = xT.rearrange("(ko p) t -> p ko t", p=P)
            ctxT_r = ctxT.rearrange("(ko p) t -> p ko t", p=P)
            wv_r = wv.rearrange("(ko p) m -> p ko m", p=P)
            wo_r = wo.rearrange("(ko p) n -> p ko n", p=P)
            mask_r = maskT.rearrange("(so p) t -> p so t", p=P)

            def _mcols(m):
                return slice(m * P, (m + 1) * P)

            # DMAs: contiguous chunks, in exact consumption order.  The
            # input stream (~9MB) runs at ~160GB/s under 8-core contention,
            # which just keeps ahead of the ~71us exp chain.
            nc.sync.dma_start(wk_sb[:, :, 0:P], wkm[0])
            for kc in range(KO):       # ctxT s-half 0 -> kt s-tiles 0..3
                nc.sync.dma_start(ctxT_sb[:, kc, 0:NT], ctxT_r[:, kc, 0:NT])
            nc.sync.dma_start(wq_sb[:, :, 0:P], wqm[0])
            for kc in range(KO):
                nc.sync.dma_start(xT_sb[:, kc, 0:NT], xT_r[:, kc, 0:NT])
            for kc in range(KO):
                nc.sync.dma_start(ctxT_sb[:, kc, NT:T], ctxT_r[:, kc, NT:T])
            for m in range(1, MQ):
                nc.sync.dma_start(wk_sb[:, :, _mcols(m)], wkm[m])
                nc.sync.dma_start(wq_sb[:, :, _mcols(m)], wqm[m])
            for kc in range(KO):
                nc.sync.dma_start(wv_sb[:, kc], wv_r[:, kc])
            for j in range(SO // 2):   # mask, t-half 0
                nc.sync.dma_start(mask_sb[:, 2 * j:2 * j + 2, 0:NT],
                                  mask_r[:, 2 * j:2 * j + 2, 0:NT])
            for kc in range(KO):
                nc.sync.dma_start(xT_sb[:, kc, NT:T], xT_r[:, kc, NT:T])
            for j in range(SO // 2):   # mask, t-half 1
                nc.sync.dma_start(mask_sb[:, 2 * j:2 * j + 2, NT:T],
                                  mask_r[:, 2 * j:2 * j + 2, NT:T])
            for kc in range(KO):
                nc.sync.dma_start(wo_sb[:, kc // 2, (kc % 2) * NT:
                                        (kc % 2) * NT + NT],
                                  wo_r[:, kc // 2, (kc % 2) * NT:
                                       (kc % 2) * NT + NT])

            # ---- projection building blocks ----
            def proj_B(m, casts_on_act=False):
                for s2 in range(T2):
                    ps = psmm.tile([P, NT], F32, tag="mm512")
                    for kc in range(KO):
                        nc.tensor.matmul(
                            ps, wk_sb[:, kc, _mcols(m)],
                            ctxT_sb[:, kc, s2 * NT:(s2 + 1) * NT],
                            start=(kc == 0), stop=(kc == KO - 1))
                    dst = kt_sb[:, m, s2 * NT:(s2 + 1) * NT]
                    if casts_on_act:
                        nc.scalar.activation(dst, ps, ACTF.Copy)
                    else:
                        nc.vector.tensor_copy(dst, ps)

            def proj_A(m, t2, casts_on_act=False):
                ps = psmm.tile([P, NT], F32, tag="mm512")
                for kc in range(KO):
                    nc.tensor.matmul(
                        ps, wq_sb[:, kc, _mcols(m)],
                        xT_sb[:, kc, t2 * NT:(t2 + 1) * NT],
                        start=(kc == 0), stop=(kc == KO - 1))
                dst = qt_sb[:, m, t2 * NT:(t2 + 1) * NT]
                if casts_on_act:
                    nc.scalar.activation(dst, ps, ACTF.Copy)
                else:
                    nc.vector.tensor_copy(dst, ps)

            def proj_V(half):        # V = ctx Wv  (natural layout), 4 s-tiles
                for so in range(half * 4, half * 4 + 4):
                    ps = psmm.tile([P, NT], F32, tag="mm512")
                    for kc in range(KO):
                        nc.tensor.matmul(
                            ps, ctxT_sb[:, kc, so * P:(so + 1) * P],
                            wv_sb[:, kc, :],
                            start=(kc == 0), stop=(kc == KO - 1))
                    nc.vector.tensor_copy(
                        vp_sb[:, so, :, 64:64 + HD],
                        ps.rearrange("p (h d) -> p h d", h=HPG))

            # ---- attention: scores for a HEAD PAIR (concurrent PE row
            # tiles at base partitions 0/64), then PV+normalize per head ----
            def scores_pair(hp, t2):
                h0, h1 = 2 * hp, 2 * hp + 1
                etA = etp.tile([P, SO, NT], BF16, tag="exp")
                etB = etp.tile([P, SO, NT], BF16, tag="exp")
                tsl = slice(t2 * NT, (t2 + 1) * NT)
                for j in range(SO // 2):   # s-tile pairs share 2-bank psums
                    psA = pssc.tile([P, 2 * NT], F32, tag="ps_sc")
                    psB = pssc.tile([P, 2 * NT], F32, tag="ps_sc")
                    for i in range(2):
                        so = 2 * j + i
                        ssl = slice(so * P, (so + 1) * P)
                        nc.tensor.matmul(
                            psA[:, i * NT:(i + 1) * NT],
                            kt_sb[_hp(h0), hp, ssl], qt_sb[_hp(h0), hp, tsl],
                            start=True, stop=True)
                        nc.tensor.matmul(
                            psB[:, i * NT:(i + 1) * NT],
                            kt_sb[_hp(h1), hp, ssl], qt_sb[_hp(h1), hp, tsl],
                            start=True, stop=True)
                    msl = mask_sb[:, 2 * j:2 * j + 2, tsl]
                    for et, ps in ((etA, psA), (etB, psB)):
                        nc.scalar.activation(
                            et[:, 2 * j:2 * j + 2, :].rearrange(
                                "p a b -> p (a b)"),
                            ps, ACTF.Exp, scale=1.0 / 32.0)
                        nc.vector.tensor_tensor(
                            et[:, 2 * j:2 * j + 2, :],
                            et[:, 2 * j:2 * j + 2, :], msl, ALU.mult)
                return etA, etB

            def pv_unit(h, t2, et):
                psu = psu_pool.tile([P, NT], F32, tag="ps_u")
                for so in range(SO):
                    nc.tensor.matmul(
                        psu, vp_sb[:, so, h, :], et[:, so, :],
                        start=(so == 0), stop=(so == SO - 1))
                rc = work.tile([1, NT], F32, tag="recip")
                nc.vector.reciprocal_approx_fast(out=rc, in_=psu[0:1, :])
                bc = work.tile([HD, NT], F32, tag="bcast")
                nc.gpsimd.partition_broadcast(bc, rc)
                nc.vector.tensor_tensor(
                    ot_sb[_hp(h), h // 2, t2 * NT:(t2 + 1) * NT],
                    psu[64:64 + HD, :], bc, ALU.mult)

            def stage_D(tm_lo, tm_hi, copies_on_act):
                for tm in range(tm_lo, tm_hi):
                    for c2 in range(C // NT):
                        ps = psmm.tile([P, NT], F32, tag="mm512")
                        for kp in range(KP):
                            nc.tensor.matmul(
                                ps, ot_sb[:, kp, tm * P:(tm + 1) * P],
                                wo_sb[:, kp, c2 * NT:(c2 + 1) * NT],
                                start=(kp == 0), stop=(kp == KP - 1))
                        o_sb = work.tile([P, NT], BF16, tag="out")
                        if copies_on_act:
                            nc.scalar.activation(o_sb, ps, ACTF.Copy)
                        else:
                            nc.vector.tensor_copy(o_sb, ps)
                        nc.sync.dma_start(
                            out[tm * P:(tm + 1) * P, c2 * NT:(c2 + 1) * NT],
                            o_sb)

            # ---- pipelined schedule (issue order = scheduler priority) ----
            ets = {}

            def S(hp, t2):
                etA, etB = scores_pair(hp, t2)
                ets[(2 * hp, t2)] = etA
                ets[(2 * hp + 1, t2)] = etB

            def PV(h, t2):
                pv_unit(h, t2, ets.pop((h, t2)))

            proj_B(0, casts_on_act=True)
            proj_A(0, 0, casts_on_act=True)
            S(0, 0)                      # heads 0,1 @ t half 0
            proj_B(1, casts_on_act=True)
            proj_A(1, 0, casts_on_act=True)
            S(1, 0)                      # heads 2,3
            proj_V(0)
            proj_V(1)
            PV(0, 0)
            PV(1, 0)
            proj_B(2)
            proj_A(2, 0)
            S(2, 0)                      # heads 4,5
            PV(2, 0); PV(3, 0)
            proj_B(3)
            proj_A(3, 0)
            S(3, 0)                      # heads 6,7
            PV(4, 0); PV(5, 0)
            proj_A(0, 1)
            proj_A(1, 1)
            S(0, 1)                      # heads 0,1 @ t half 1
            PV(6, 0); PV(7, 0)
            stage_D(0, 2, False)
            proj_A(2, 1)
            proj_A(3, 1)
            S(1, 1)                      # heads 2,3
            PV(0, 1); PV(1, 1)
            stage_D(2, 4, False)
            S(2, 1)                      # heads 4,5
            PV(2, 1); PV(3, 1)
            S(3, 1)                      # heads 6,7
            PV(4, 1); PV(5, 1)
            PV(6, 1); PV(7, 1)
            stage_D(TM_HALF, 2 * TM_HALF, True)
    nc.compile()
    return nc


def _get_program():
    global _CACHED_NC
    if _CACHED_NC is None:
        _CACHED_NC = _build_program()
    return _CACHED_NC


def kernel(x, context, attn_mask, Wq, Wk, Wv, Wo):
    x = np.asarray(x, dtype=np.float32)
    context = np.asarray(context, dtype=np.float32)
    attn_mask = np.asarray(attn_mask)
    Wq = np.asarray(Wq, dtype=np.float32)
    Wk = np.asarray(Wk, dtype=np.float32)
    Wv = np.asarray(Wv, dtype=np.float32)
    Wo = np.asarray(Wo, dtype=np.float32)

    nc = _get_program()
    bf = ml_dtypes.bfloat16

    def _mchunk(w):
        # [C, CG_slice] -> [MQ, P, KO, P]: per column-block, partition-major
        return np.ascontiguousarray(
            w.reshape(KO, P, MQ, P).transpose(2, 1, 0, 3)).astype(bf)

    in_maps = []
    for i in range(NCORES):
        b, g = i // 2, i % 2
        cs = slice(g * CG, (g + 1) * CG)
        in_maps.append({
            "xT": np.ascontiguousarray(x[b].T).astype(bf),
            "ctxT": np.ascontiguousarray(context[b].T).astype(bf),
            "maskT": np.ascontiguousarray(attn_mask[b, 0].T).astype(bf),
            "wqm": _mchunk(Wq[:, cs]),
            "wkm": _mchunk(Wk[:, cs]),
            "wv": np.ascontiguousarray(Wv[:, cs]).astype(bf),
            "wo": np.ascontiguousarray(Wo[cs, :]).astype(bf),
        })

    profile = os.environ.get("KERNEL_PROFILE", "0") == "1"
    if profile:
        _ensure_ntff_hook()
    res = run_bass_kernel_spmd(
        nc, in_maps, list(range(NCORES)),
        trace=profile, trace_cores=[0] if profile else None)
    if profile:
        kernel.last_exec_time_ns = res.exec_time_ns
        kernel.last_trace = res.instructions_and_trace

    out = np.empty((B, T, C), dtype=np.float32)
    for b in range(B):
        out[b] = (res.results[2 * b]["out"].astype(np.float32)
                  + res.results[2 * b + 1]["out"].astype(np.float32))
    return out
